# revision 1
# baseline (speedup 1.0000x reference)
"""Trainium2 Bass kernel for the BDH dense-transformer problem.

Sharding: data-parallel over B=8 across the 8 NeuronCores (one batch
element per core, no collectives). Each core runs the full 6-layer
network on its [T=2048, D=256] slice.

Per-core program. Matmul precision strategy (HW-measured):
  - attention / embedding / readout matmuls: fp32 (4 cyc/row)
  - MLP x/y branches AND update matmul (the FLOP bulk): bf16x2
    split-precision -- weights split hi+lo bf16 on host, activations
    split on DVE, 3 accumulating bf16 passes (hi*hi + hi*lo + lo*hi,
    3 cyc/row) ~= fp32 precision. End-to-end rel err 3.3e-4 on HW
    (vs 6e-5 all-fp32 at 9.2ms modeled, vs 1.5e-2 float32r); this
    version models 7.5ms
Structure:
  - token embedding via one-hot matmul (iota + is_equal + PE)
  - v kept in both layouts: vT [D,T] and vN [T,D]
  - causal linear attention block-wise: energyT = qr@qr^T per
    [s128, t512] block (PSUM), bf16-mask multiply, then aN accumulated
    in PSUM over s-chunks
  - LayerNorms in natural layout with fused ACT Square/Identity
    (per-partition scale+bias + accum_out row sums)
  - MLP streamed over N in quarters (weights DMA'd per layer in
    host-pre-shuffled partition-contiguous layouts),
    relu(x)*relu(y) fused via scalar_tensor_tensor, update accumulated
    in PSUM then SBUF (updA aliases qrT storage - disjoint lifetimes)
  - PE 128x128 transposes maintain both v layouts
"""

import math

import numpy as np
import ml_dtypes

import concourse.bass as bass
import concourse.tile as tile
from concourse import bacc, mybir
from concourse import bass_utils

F32 = mybir.dt.float32
F32R = mybir.dt.float32r
BF16 = mybir.dt.bfloat16
I32 = mybir.dt.int32
ALU = mybir.AluOpType
ACTF = mybir.ActivationFunctionType
AXX = mybir.AxisListType.X

B, T, D, N, H, VOCAB, L = 8, 2048, 256, 8192, 4, 256, 6
EPS = 1e-5
TS = 512          # t-super width
NSUP = T // TS    # 4
NTB = T // 128    # 16
NQ = 4            # weight quarters along N
NCHQ = N // 128 // NQ  # 16 n-chunks per quarter


USE_F32R = False
WDT = F32R if USE_F32R else F32


def r(ap):
    if not USE_F32R:
        return ap
    return ap.bitcast(F32R)


def build_nc(layers=L, stream_weights=True, attn=True, cphase=True):
    nc = bacc.Bacc("TRN2", target_bir_lowering=False, debug=False)

    idx_d = nc.dram_tensor("idxf", [1, T], WDT, kind="ExternalInput")
    wte_d = nc.dram_tensor("wte", [VOCAB, D], F32, kind="ExternalInput")
    wxh_d = nc.dram_tensor("wxh", [128, 2, N], BF16, kind="ExternalInput")
    wxl_d = nc.dram_tensor("wxl", [128, 2, N], BF16, kind="ExternalInput")
    wyh_d = nc.dram_tensor("wyh", [128, 2, N], BF16, kind="ExternalInput")
    wyl_d = nc.dram_tensor("wyl", [128, 2, N], BF16, kind="ExternalInput")
    ench_d = nc.dram_tensor("ench", [128, N // 128, D], BF16, kind="ExternalInput")
    encl_d = nc.dram_tensor("encl", [128, N // 128, D], BF16, kind="ExternalInput")
    ro_d = nc.dram_tensor("ro", [D, VOCAB], WDT, kind="ExternalInput")
    cos_d = nc.dram_tensor("cosT", [128, T], F32, kind="ExternalInput")
    sin_d = nc.dram_tensor("sinT", [128, T], F32, kind="ExternalInput")
    mask_d = nc.dram_tensor("maskbig", [128, 1024], BF16, kind="ExternalInput")
    ident_d = nc.dram_tensor("identm", [128, 128], F32, kind="ExternalInput")
    out_d = nc.dram_tensor("logits", [T, VOCAB], F32, kind="ExternalOutput")

    wxh_r, wxl_r, wyh_r, wyl_r = wxh_d.ap(), wxl_d.ap(), wyh_d.ap(), wyl_d.ap()
    ench_r, encl_r = ench_d.ap(), encl_d.ap()
    wte_r = wte_d.ap().rearrange("(c p) d -> p c d", p=128)
    ro_r = ro_d.ap().rearrange("(c p) d -> p c d", p=128)

    with tile.TileContext(nc) as tc:
        with tc.tile_pool(name="persist", bufs=1) as pp, \
             tc.tile_pool(name="wq", bufs=2) as wq, \
             tc.tile_pool(name="blk", bufs=4) as blkp, \
             tc.tile_pool(name="sc", bufs=7) as scp, \
             tc.tile_pool(name="st", bufs=32) as stp, \
             tc.tile_pool(name="bfp", bufs=12) as bfp, \
             tc.tile_pool(name="ps512", bufs=4, space="PSUM") as ps512, \
             tc.tile_pool(name="ps256", bufs=4, space="PSUM") as ps256:

            vT = [pp.tile([128, T], WDT, name=f"vT{c}", tag=f"vT{c}") for c in range(2)]
            vN = pp.tile([128, NTB, D], F32, name="vN", tag="vN")
            qrT = [pp.tile([128, T], F32, name=f"qrT{c}", tag=f"qrT{c}") for c in range(2)]
            lnaT = [pp.tile([128, T], WDT, name=f"lnaT{c}", tag=f"lnaT{c}") for c in range(2)]
            # updA aliases qrT: disjoint lifetimes within a layer (qrT read only
            # in phase A; updA written in phase B, read in phase C before next rope)
            _updv = [q.rearrange("p (o d) -> p o d", d=D) for q in qrT]

            def updA(tb):
                return _updv[tb // 8][:, tb % 8, :]
            cosT = pp.tile([128, T], F32, name="cosT", tag="cosT")
            sinT = pp.tile([128, T], F32, name="sinT", tag="sinT")
            maskb = pp.tile([128, 1024], BF16, name="maskb", tag="maskb")

            ident = pp.tile([128, 128], F32, name="ident", tag="ident")
            iota_f = pp.tile([128, 2], F32, name="iota_f", tag="iota_f")

            nc.sync.dma_start(cosT[:], cos_d.ap())
            nc.sync.dma_start(sinT[:], sin_d.ap())
            nc.sync.dma_start(maskb[:], mask_d.ap())

            nc.sync.dma_start(ident[:], ident_d.ap())

            copy_flip = [0]

            def copy_any(dst, src):
                # alternate PSUM->SBUF copies between ACT and DVE
                copy_flip[0] ^= 1
                if copy_flip[0]:
                    nc.scalar.copy(dst, src)
                else:
                    nc.vector.tensor_copy(dst, src)

            def mm(psum, lhsT, rhs, start, stop):
                nc.tensor.matmul(psum, r(lhsT), r(rhs), start=start, stop=stop)

            def mm32(psum, lhsT, rhs, start, stop):
                nc.tensor.matmul(psum, lhsT, rhs, start=start, stop=stop)

            def tr128(dst, src):
                pst = ps512.tile([128, 512], F32, name="pst", tag="ps512")
                nc.tensor.transpose(pst[:, :128], src, ident[:])
                copy_any(dst, pst[:, :128])

            def ln_nat(src, dst, sums=None):
                """LayerNorm over free dim (256) of [128, 256] src -> dst.

                src may be PSUM or SBUF. sums = optional precomputed row sums.
                """
                if sums is None:
                    sums = stp.tile([128, 1], F32, name="s1", tag="st")
                    nc.vector.reduce_sum(sums, src, axis=AXX)
                negmean = stp.tile([128, 1], F32, name="negmean", tag="st")
                nc.vector.tensor_scalar_mul(negmean, sums, -1.0 / D)
                sq = scp.tile([128, D], F32, name="sq", tag="sc")
                sqs = stp.tile([128, 1], F32, name="sqs", tag="st")
                nc.scalar.activation(sq, src, ACTF.Square, bias=negmean, scale=1.0,
                                     accum_out=sqs)
                veps = stp.tile([128, 1], F32, name="veps", tag="st")
                nc.vector.tensor_scalar(veps, sqs, 1.0 / D, EPS, op0=ALU.mult, op1=ALU.add)
                sqv = stp.tile([128, 1], F32, name="sqv", tag="st")
                nc.scalar.sqrt(sqv, veps)
                rstd = stp.tile([128, 1], F32, name="rstd", tag="st")
                nc.vector.reciprocal(rstd, sqv)
                negmurs = stp.tile([128, 1], F32, name="negmurs", tag="st")
                nc.vector.tensor_tensor(negmurs, negmean, rstd, op=ALU.mult)
                nc.scalar.activation(dst, src, ACTF.Identity, bias=negmurs, scale=rstd)

            # ---------------- embedding: v = ln(wte[idx]) ----------------
            iota_i = pp.tile([128, 2], I32, name="iota_i", tag="iota_i")
            for c in range(2):
                nc.gpsimd.iota(iota_i[:, c:c + 1], pattern=[[1, 1]], base=c * 128,
                               channel_multiplier=1)
            nc.vector.tensor_copy(iota_f[:], iota_i[:])
            idx_b = lnaT[0]  # scratch alias
            nc.sync.dma_start(idx_b[:], idx_d.ap().partition_broadcast(128))
            for c in range(2):
                # one-hot^T chunk in qrT[c] (scratch alias)
                nc.vector.tensor_scalar(qrT[c][:], idx_b[:], iota_f[:, c:c + 1], None,
                                        op0=ALU.is_equal)
            wte_s = blkp.tile([128, 2, D], F32, name="wte_s", tag="blk")
            nc.sync.dma_start(wte_s[:], wte_r)
            for tb in range(NTB):
                psA = ps256.tile([128, D], F32, name="psE", tag="ps256")
                for c in range(2):
                    mm32(psA, qrT[c][:, tb * 128:(tb + 1) * 128], wte_s[:, c, :],
                         start=(c == 0), stop=(c == 1))
                ln_nat(psA, vN[:, tb, :])
                for c in range(2):
                    tr128(vT[c][:, tb * 128:(tb + 1) * 128], vN[:, tb, c * 128:(c + 1) * 128])

            # ---------------- layers ----------------
            if not stream_weights:
                wxqh0 = wq.tile([128, 2, N // NQ], BF16, name="wxqh", tag="wxqh")
                nc.sync.dma_start(wxqh0[:], wxh_r[:, :, 0:N // NQ])
                wxql0 = wq.tile([128, 2, N // NQ], BF16, name="wxql", tag="wxql")
                nc.sync.dma_start(wxql0[:], wxl_r[:, :, 0:N // NQ])
                wyqh0 = wq.tile([128, 2, N // NQ], BF16, name="wyqh", tag="wyqh")
                nc.sync.dma_start(wyqh0[:], wyh_r[:, :, 0:N // NQ])
                wyql0 = wq.tile([128, 2, N // NQ], BF16, name="wyql", tag="wyql")
                nc.sync.dma_start(wyql0[:], wyl_r[:, :, 0:N // NQ])
                encqh0 = wq.tile([128, NCHQ, D], BF16, name="encqh", tag="encqh")
                nc.sync.dma_start(encqh0[:], ench_r[:, 0:NCHQ, :])
                encql0 = wq.tile([128, NCHQ, D], BF16, name="encql", tag="encql")
                nc.sync.dma_start(encql0[:], encl_r[:, 0:NCHQ, :])
            for layer in range(layers):
                # --- rope: qrT = vT*cos +/- rot*sin ---
                rsc = lnaT[1]  # dead scratch at this point
                nc.vector.tensor_tensor(qrT[0][:], vT[0][:], cosT[:], op=ALU.mult)
                nc.vector.tensor_tensor(rsc[:], vT[1][:], sinT[:], op=ALU.mult)
                nc.vector.tensor_tensor(qrT[0][:], qrT[0][:], rsc[:], op=ALU.subtract)
                nc.vector.tensor_tensor(qrT[1][:], vT[1][:], cosT[:], op=ALU.mult)
                nc.vector.tensor_tensor(rsc[:], vT[0][:], sinT[:], op=ALU.mult)
                nc.vector.tensor_tensor(qrT[1][:], qrT[1][:], rsc[:], op=ALU.add)

                # --- attention + LN(a) -> lnaT ---
                for si in range(NSUP if attn else 0):
                    psA = [ps256.tile([128, D], F32, name="psA", tag="ps256")
                           for _ in range(4)]
                    for sc in range(4 * si + 4):
                        psE = ps512.tile([128, TS], F32, name="psE", tag="ps512")
                        for c in range(2):
                            mm32(psE, qrT[c][:, sc * 128:(sc + 1) * 128],
                                 qrT[c][:, si * TS:(si + 1) * TS],
                                 start=(c == 0), stop=(c == 1))
                        eT = blkp.tile([128, TS], F32, name="eT", tag="blk")
                        k = sc - 4 * si
                        if k < 0:
                            copy_any(eT[:], psE[:])
                        else:
                            nc.vector.tensor_tensor(
                                eT[:], psE[:], maskb[:, 384 - k * 128: 896 - k * 128],
                                op=ALU.mult)
                        for tb4 in range(4):
                            tb = si * 4 + tb4
                            if sc <= tb:
                                mm32(psA[tb4], eT[:, tb4 * 128:(tb4 + 1) * 128],
                                     vN[:, sc, :], start=(sc == 0), stop=(sc == tb))
                    for tb4 in range(4):
                        tb = si * 4 + tb4
                        lna_n = scp.tile([128, D], F32, name="lna_n", tag="sc")
                        ln_nat(psA[tb4], lna_n)
                        for c in range(2):
                            tr128(lnaT[c][:, tb * 128:(tb + 1) * 128],
                                  lna_n[:, c * 128:(c + 1) * 128])

                # --- MLP over N quarters ---
                upd_sums = {}
                for q in range(NQ):
                    if stream_weights:
                        qs = slice(q * (N // NQ), (q + 1) * (N // NQ))
                        wxqh = wq.tile([128, 2, N // NQ], BF16, name="wxqh", tag="wxqh")
                        nc.sync.dma_start(wxqh[:], wxh_r[:, :, qs])
                        wxql = wq.tile([128, 2, N // NQ], BF16, name="wxql", tag="wxql")
                        nc.sync.dma_start(wxql[:], wxl_r[:, :, qs])
                        wyqh = wq.tile([128, 2, N // NQ], BF16, name="wyqh", tag="wyqh")
                        nc.sync.dma_start(wyqh[:], wyh_r[:, :, qs])
                        wyql = wq.tile([128, 2, N // NQ], BF16, name="wyql", tag="wyql")
                        nc.sync.dma_start(wyql[:], wyl_r[:, :, qs])
                        encqh = wq.tile([128, NCHQ, D], BF16, name="encqh", tag="encqh")
                        nc.sync.dma_start(encqh[:], ench_r[:, q * NCHQ:(q + 1) * NCHQ, :])
                        encql = wq.tile([128, NCHQ, D], BF16, name="encql", tag="encql")
                        nc.sync.dma_start(encql[:], encl_r[:, q * NCHQ:(q + 1) * NCHQ, :])
                    else:
                        wxqh, wxql, wyqh, wyql = wxqh0, wxql0, wyqh0, wyql0
                        encqh, encql = encqh0, encql0
                    for si in range(NSUP):
                        sl = slice(si * TS, (si + 1) * TS)
                        ln_src = lnaT if attn else qrT
                        vhl, lhl = [], []
                        for c in range(2):
                            vh = bfp.tile([128, TS], BF16, name="vh", tag="bfp")
                            nc.vector.tensor_copy(vh[:], vT[c][:, sl])
                            vl = bfp.tile([128, TS], BF16, name="vl", tag="bfp")
                            nc.vector.tensor_tensor(vl[:], vT[c][:, sl], vh[:],
                                                    op=ALU.subtract)
                            vhl.append((vh, vl))
                            lh = bfp.tile([128, TS], BF16, name="lh", tag="bfp")
                            nc.vector.tensor_copy(lh[:], ln_src[c][:, sl])
                            ll = bfp.tile([128, TS], BF16, name="ll", tag="bfp")
                            nc.vector.tensor_tensor(ll[:], ln_src[c][:, sl], lh[:],
                                                    op=ALU.subtract)
                            lhl.append((lh, ll))
                        psU = [ps256.tile([128, D], F32, name="psU", tag="ps256")
                               for _ in range(4)]
                        for nch in range(NCHQ):
                            psX = ps512.tile([128, TS], F32, name="psX", tag="ps512")
                            psY = ps512.tile([128, TS], F32, name="psY", tag="ps512")
                            ns = slice(nch * 128, (nch + 1) * 128)
                            for i, (wqh, wql, act) in enumerate(
                                    ((wxqh, wxql, vhl), (wyqh, wyql, lhl))):
                                ps = psX if i == 0 else psY
                                terms = []
                                for c in range(2):
                                    ah, al = act[c]
                                    terms += [(wqh[:, c, ns], ah), (wqh[:, c, ns], al),
                                              (wql[:, c, ns], ah)]
                                for j, (w_ap, a_t) in enumerate(terms):
                                    nc.tensor.matmul(ps, w_ap, a_t[:],
                                                     start=(j == 0),
                                                     stop=(j == len(terms) - 1))
                            xr = blkp.tile([128, TS], F32, name="xr", tag="blk")
                            nc.scalar.activation(xr, psX, ACTF.Relu)
                            ysb = blkp.tile([128, TS], F32, name="ysb", tag="blk")
                            nc.vector.scalar_tensor_tensor(
                                ysb, psY, 0.0, xr, op0=ALU.max, op1=ALU.mult)
                            ysh = bfp.tile([128, TS], BF16, name="ysh", tag="bfp")
                            nc.vector.tensor_copy(ysh[:], ysb[:])
                            ysl = bfp.tile([128, TS], BF16, name="ysl", tag="bfp")
                            nc.vector.tensor_tensor(ysl[:], ysb[:], ysh[:],
                                                    op=ALU.subtract)
                            for tb4 in range(4):
                                t4 = slice(tb4 * 128, (tb4 + 1) * 128)
                                for j, (ya, ea) in enumerate(
                                        ((ysh, encqh), (ysl, encqh), (ysh, encql))):
                                    nc.tensor.matmul(
                                        psU[tb4], ya[:, t4], ea[:, nch, :],
                                        start=(nch == 0 and j == 0),
                                        stop=(nch == NCHQ - 1 and j == 2))
                        for tb4 in range(4):
                            tb = si * 4 + tb4
                            dst = updA(tb)
                            if q == 0:
                                nc.scalar.copy(dst, psU[tb4])
                            elif q < NQ - 1:
                                nc.vector.tensor_tensor(dst, psU[tb4], dst, op=ALU.add)
                            else:
                                s2 = stp.tile([128, 1], F32, name="s2", tag="st")
                                nc.vector.scalar_tensor_tensor(
                                    dst, psU[tb4], 0.0, dst, op0=ALU.add, op1=ALU.add,
                                    accum_out=s2)
                                upd_sums[tb] = s2

                # --- v = ln(v + ln(update)); maintain both layouts ---
                for tb in range(NTB if cphase else 0):
                    upd = updA(tb)
                    lnu = scp.tile([128, D], F32, name="lnu", tag="sc")
                    ln_nat(upd, lnu, sums=upd_sums[tb])
                    vmid = scp.tile([128, D], F32, name="vmid", tag="sc")
                    s3 = stp.tile([128, 1], F32, name="s3", tag="st")
                    nc.vector.scalar_tensor_tensor(
                        vmid, lnu, 0.0, vN[:, tb, :], op0=ALU.add, op1=ALU.add,
                        accum_out=s3)
                    ln_nat(vmid, vN[:, tb, :], sums=s3)
                    for c in range(2):
                        tr128(vT[c][:, tb * 128:(tb + 1) * 128],
                              vN[:, tb, c * 128:(c + 1) * 128])

            # ---------------- readout ----------------
            ro_s = blkp.tile([128, 2, D], WDT, name="ro_s", tag="blk")
            nc.sync.dma_start(ro_s[:], ro_r)
            for tb in range(NTB):
                psR = ps256.tile([128, D], F32, name="psR", tag="ps256")
                for c in range(2):
                    mm(psR, vT[c][:, tb * 128:(tb + 1) * 128], ro_s[:, c, :],
                       start=(c == 0), stop=(c == 1))
                lo = scp.tile([128, VOCAB], F32, name="lo", tag="sc")
                copy_any(lo[:], psR[:])
                nc.sync.dma_start(out_d.ap()[tb * 128:(tb + 1) * 128, :], lo[:])

    nc.compile()
    return nc


_NC_CACHE = {}


def get_nc():
    if "nc" not in _NC_CACHE:
        _NC_CACHE["nc"] = build_nc()
    return _NC_CACHE["nc"]


def make_host_inputs(idx, wte, encoder, decoder_x, decoder_y, readout):
    idx = np.asarray(idx)
    wte = np.asarray(wte, dtype=np.float32)
    encoder = np.asarray(encoder, dtype=np.float32)
    decoder_x = np.asarray(decoder_x, dtype=np.float32)
    decoder_y = np.asarray(decoder_y, dtype=np.float32)
    readout = np.asarray(readout, dtype=np.float32)

    wx = decoder_x.transpose(1, 0, 2).reshape(D, N)
    wy = decoder_y.transpose(1, 0, 2).reshape(D, N)
    # partition-contiguous layouts for fast DMA: [p, c, n] with d = c*128 + p
    wx = np.ascontiguousarray(wx.reshape(2, 128, N).transpose(1, 0, 2))
    wy = np.ascontiguousarray(wy.reshape(2, 128, N).transpose(1, 0, 2))
    # bf16x2 split: w = hi + lo with hi = bf16(w), lo = bf16(w - hi)
    wxh = wx.astype(ml_dtypes.bfloat16)
    wxl = (wx - wxh.astype(np.float32)).astype(ml_dtypes.bfloat16)
    wyh = wy.astype(ml_dtypes.bfloat16)
    wyl = (wy - wyh.astype(np.float32)).astype(ml_dtypes.bfloat16)
    # enc: [p, o, d] with n = o*128 + p
    enc_s = np.ascontiguousarray(encoder.reshape(N // 128, 128, D).transpose(1, 0, 2))
    ench = enc_s.astype(ml_dtypes.bfloat16)
    encl = (enc_s - ench.astype(np.float32)).astype(ml_dtypes.bfloat16)

    inv_freq = 1.0 / (10000.0 ** (np.arange(0, D, 2, dtype=np.float32) / D))  # [128]
    t = np.arange(T, dtype=np.float32)
    freqsT = inv_freq[:, None] * t[None, :]                   # [128, T]
    cosT = np.cos(freqsT).astype(np.float32)
    sinT = np.sin(freqsT).astype(np.float32)

    s_idx = np.arange(128, dtype=np.int32)[:, None]
    c_idx = np.arange(1024, dtype=np.int32)[None, :]
    maskbig = (s_idx <= c_idx - 384).astype(ml_dtypes.bfloat16)

    in_maps = []
    for b in range(B):
        in_maps.append({
            "idxf": idx[b].astype(np.float32).reshape(1, T),
            "wte": wte,
            "wxh": wxh,
            "wxl": wxl,
            "wyh": wyh,
            "wyl": wyl,
            "ench": ench,
            "encl": encl,
            "ro": readout,
            "cosT": cosT,
            "sinT": sinT,
            "maskbig": maskbig,
            "identm": np.eye(128, dtype=np.float32),
        })
    return in_maps


def kernel(idx, wte, encoder, decoder_x, decoder_y, readout):
    nc = get_nc()
    in_maps = make_host_inputs(idx, wte, encoder, decoder_x, decoder_y, readout)
    res = bass_utils.run_bass_kernel_spmd(nc, in_maps, core_ids=list(range(B)))
    out = np.stack([res.results[b]["logits"] for b in range(B)], axis=0)
    return out.astype(np.float32)



# revision 5
# speedup vs baseline: 2.5536x; 2.5536x over previous
"""Trainium2 Bass kernel for the BDH dense-transformer problem.

Sharding: data-parallel over B=8 across the 8 NeuronCores (one batch
element per core, no collectives). Each core runs the full 6-layer
network on its [T=2048, D=256] slice.

Per-core program. All matmuls run in single-pass float32r (1 cyc/row on
PE for output free size >= 256; HW-measured effective mantissa ~11 bits,
per-matmul rel err ~1.5e-4). Tensors feeding f32r matmuls are declared
float32r so producer instructions emit f32r-rounded outputs (BIR
verifier requirement); DMA-fed weights stay unrounded fp32 bits, which
HW-measurably matches DVE-rounded operands (PE rounds internally).

Structure:
  - token embedding via one-hot matmul (iota + is_equal + PE)
  - v kept in both layouts: vT [D,T] and vN [T,D]
  - causal linear attention block-wise: energyT = qr@qr^T per
    [s128, t512] block (PSUM), bf16-mask multiply, then aN accumulated
    in PSUM over s-chunks
  - LayerNorms in natural layout with fused ACT Square/Identity
    (per-partition scale+bias + accum_out row sums)
  - MLP streamed over N in quarters (weights DMA'd per layer in
    host-pre-shuffled partition-contiguous layouts),
    relu(x)*relu(y) fused via scalar_tensor_tensor, update accumulated
    in PSUM then SBUF (updA aliases qrT storage - disjoint lifetimes)
  - PE 128x128 transposes maintain both v layouts
"""

import math

import numpy as np
import ml_dtypes

import concourse.bass as bass
import concourse.tile as tile
from concourse import bacc, mybir
from concourse import bass_utils

F32 = mybir.dt.float32
F32R = mybir.dt.float32r
BF16 = mybir.dt.bfloat16
I32 = mybir.dt.int32
ALU = mybir.AluOpType
ACTF = mybir.ActivationFunctionType
AXX = mybir.AxisListType.X

B, T, D, N, H, VOCAB, L = 8, 2048, 256, 8192, 4, 256, 6
EPS = 1e-5
TS = 512          # t-super width
NSUP = T // TS    # 4
NTB = T // 128    # 16
NQ = 4            # weight quarters along N
NCHQ = N // 128 // NQ  # 16 n-chunks per quarter


def build_nc(layers=L, attn=True, cphase=True):
    nc = bacc.Bacc("TRN2", target_bir_lowering=False, debug=False)

    idx_d = nc.dram_tensor("idxf", [1, T], F32R, kind="ExternalInput")
    wte_d = nc.dram_tensor("wte", [VOCAB, D], F32R, kind="ExternalInput")
    wx_d = nc.dram_tensor("wx", [128, 2, N], F32R, kind="ExternalInput")
    wy_d = nc.dram_tensor("wy", [128, 2, N], F32R, kind="ExternalInput")
    enc_d = nc.dram_tensor("enc", [128, N // 128, D], F32R, kind="ExternalInput")
    ro_d = nc.dram_tensor("ro", [D, VOCAB], F32R, kind="ExternalInput")
    cos_d = nc.dram_tensor("cosT", [128, T], F32, kind="ExternalInput")
    sin_d = nc.dram_tensor("sinT", [128, T], F32, kind="ExternalInput")
    mask_d = nc.dram_tensor("maskbig", [128, 1024], BF16, kind="ExternalInput")
    ident_d = nc.dram_tensor("identm", [128, 128], F32, kind="ExternalInput")
    out_d = nc.dram_tensor("logits", [T, VOCAB], F32, kind="ExternalOutput")

    wx_r, wy_r, enc_r = wx_d.ap(), wy_d.ap(), enc_d.ap()
    wte_r = wte_d.ap().rearrange("(c p) d -> p c d", p=128)
    ro_r = ro_d.ap().rearrange("(c p) d -> p c d", p=128)

    with tile.TileContext(nc) as tc:
        with tc.tile_pool(name="persist", bufs=1) as pp, \
             tc.tile_pool(name="wq", bufs=2) as wq, \
             tc.tile_pool(name="blk", bufs=4) as blkp, \
             tc.tile_pool(name="sc", bufs=7) as scp, \
             tc.tile_pool(name="st", bufs=32) as stp, \
             tc.tile_pool(name="ps512", bufs=4, space="PSUM") as ps512, \
             tc.tile_pool(name="ps256", bufs=4, space="PSUM") as ps256:

            vT = [pp.tile([128, T], F32R, name=f"vT{c}", tag=f"vT{c}") for c in range(2)]
            vN = pp.tile([128, NTB, D], F32R, name="vN", tag="vN")
            qrT = [pp.tile([128, T], F32R, name=f"qrT{c}", tag=f"qrT{c}") for c in range(2)]
            lnaT = [pp.tile([128, T], F32R, name=f"lnaT{c}", tag=f"lnaT{c}") for c in range(2)]
            # updA aliases qrT: disjoint lifetimes within a layer (qrT read only
            # in phase A; updA written in phase B, read in phase C before next
            # rope fully overwrites qrT). The BIR verifier checks rounding per
            # memory location, so updA partial sums are f32r-rounded too
            # (~2^-12 extra relative noise per accumulation step).
            _updv = [q.rearrange("p (o d) -> p o d", d=D) for q in qrT]

            def updA(tb):
                return _updv[tb // 8][:, tb % 8, :]
            cosT = pp.tile([128, T], F32, name="cosT", tag="cosT")
            sinT = pp.tile([128, T], F32, name="sinT", tag="sinT")
            maskb = pp.tile([128, 1024], BF16, name="maskb", tag="maskb")

            ident = pp.tile([128, 128], F32, name="ident", tag="ident")
            iota_f = pp.tile([128, 2], F32, name="iota_f", tag="iota_f")

            nc.sync.dma_start(cosT[:], cos_d.ap())
            nc.sync.dma_start(sinT[:], sin_d.ap())
            nc.sync.dma_start(maskb[:], mask_d.ap())

            nc.sync.dma_start(ident[:], ident_d.ap())

            copy_flip = [0]

            def copy_any(dst, src):
                # alternate PSUM->SBUF copies between ACT and DVE
                copy_flip[0] ^= 1
                if copy_flip[0]:
                    nc.scalar.copy(dst, src)
                else:
                    nc.vector.tensor_copy(dst, src)

            def tr128(dst, src):
                pst = ps512.tile([128, 512], F32, name="pst", tag="ps512")
                if src.dtype != F32:
                    src = src.bitcast(F32)
                nc.tensor.transpose(pst[:, :128], src, ident[:])
                copy_any(dst, pst[:, :128])

            def ln_nat(src, dst, sums=None):
                """LayerNorm over free dim (256) of [128, 256] src -> dst.

                src may be PSUM or SBUF. sums = optional precomputed row sums.
                """
                if sums is None:
                    sums = stp.tile([128, 1], F32, name="s1", tag="st")
                    nc.vector.reduce_sum(sums, src, axis=AXX)
                negmean = stp.tile([128, 1], F32, name="negmean", tag="st")
                nc.vector.tensor_scalar_mul(negmean, sums, -1.0 / D)
                sq = scp.tile([128, D], F32, name="sq", tag="sc")
                sqs = stp.tile([128, 1], F32, name="sqs", tag="st")
                nc.scalar.activation(sq, src, ACTF.Square, bias=negmean, scale=1.0,
                                     accum_out=sqs)
                veps = stp.tile([128, 1], F32, name="veps", tag="st")
                nc.vector.tensor_scalar(veps, sqs, 1.0 / D, EPS, op0=ALU.mult, op1=ALU.add)
                sqv = stp.tile([128, 1], F32, name="sqv", tag="st")
                nc.scalar.sqrt(sqv, veps)
                rstd = stp.tile([128, 1], F32, name="rstd", tag="st")
                nc.vector.reciprocal(rstd, sqv)
                negmurs = stp.tile([128, 1], F32, name="negmurs", tag="st")
                nc.vector.tensor_tensor(negmurs, negmean, rstd, op=ALU.mult)
                nc.scalar.activation(dst, src, ACTF.Identity, bias=negmurs, scale=rstd)

            # ---------------- embedding: v = ln(wte[idx]) ----------------
            iota_i = pp.tile([128, 2], I32, name="iota_i", tag="iota_i")
            for c in range(2):
                nc.gpsimd.iota(iota_i[:, c:c + 1], pattern=[[1, 1]], base=c * 128,
                               channel_multiplier=1)
            nc.vector.tensor_copy(iota_f[:], iota_i[:])
            idx_b = lnaT[0]  # scratch alias
            nc.sync.dma_start(idx_b[:], idx_d.ap().partition_broadcast(128))
            for c in range(2):
                # one-hot^T chunk in qrT[c] (scratch alias); 0/1 exact in f32r
                nc.vector.tensor_scalar(qrT[c][:], idx_b[:], iota_f[:, c:c + 1], None,
                                        op0=ALU.is_equal)
            wte_s = blkp.tile([128, 2, D], F32R, name="wte_s", tag="blk")
            nc.sync.dma_start(wte_s[:], wte_r)
            for tb in range(NTB):
                psA = ps256.tile([128, D], F32, name="psE", tag="ps256")
                for c in range(2):
                    nc.tensor.matmul(psA, qrT[c][:, tb * 128:(tb + 1) * 128],
                                     wte_s[:, c, :], start=(c == 0), stop=(c == 1))
                ln_nat(psA, vN[:, tb, :])
                for c in range(2):
                    tr128(vT[c][:, tb * 128:(tb + 1) * 128], vN[:, tb, c * 128:(c + 1) * 128])

            # ---------------- layers ----------------
            for layer in range(layers):
                # --- rope: qrT = vT*cos +/- rot*sin ---
                rsc = lnaT[1]  # dead scratch at this point
                nc.vector.tensor_tensor(qrT[0][:], vT[0][:], cosT[:], op=ALU.mult)
                nc.vector.tensor_tensor(rsc[:], vT[1][:], sinT[:], op=ALU.mult)
                nc.vector.tensor_tensor(qrT[0][:], qrT[0][:], rsc[:], op=ALU.subtract)
                nc.vector.tensor_tensor(qrT[1][:], vT[1][:], cosT[:], op=ALU.mult)
                nc.vector.tensor_tensor(rsc[:], vT[0][:], sinT[:], op=ALU.mult)
                nc.vector.tensor_tensor(qrT[1][:], qrT[1][:], rsc[:], op=ALU.add)

                # --- attention + LN(a) -> lnaT ---
                for si in range(NSUP if attn else 0):
                    psA = [ps256.tile([128, D], F32, name="psA", tag="ps256")
                           for _ in range(4)]
                    for sc in range(4 * si + 4):
                        psE = ps512.tile([128, TS], F32, name="psE", tag="ps512")
                        for c in range(2):
                            nc.tensor.matmul(psE, qrT[c][:, sc * 128:(sc + 1) * 128],
                                             qrT[c][:, si * TS:(si + 1) * TS],
                                             start=(c == 0), stop=(c == 1))
                        eT = blkp.tile([128, TS], F32R, name="eT", tag="blk")
                        k = sc - 4 * si
                        if k < 0:
                            copy_any(eT[:], psE[:])
                        else:
                            nc.vector.tensor_tensor(
                                eT[:], psE[:], maskb[:, 384 - k * 128: 896 - k * 128],
                                op=ALU.mult)
                        for tb4 in range(4):
                            tb = si * 4 + tb4
                            if sc <= tb:
                                nc.tensor.matmul(psA[tb4],
                                                 eT[:, tb4 * 128:(tb4 + 1) * 128],
                                                 vN[:, sc, :], start=(sc == 0),
                                                 stop=(sc == tb))
                    for tb4 in range(4):
                        tb = si * 4 + tb4
                        lna_n = scp.tile([128, D], F32, name="lna_n", tag="sc")
                        ln_nat(psA[tb4], lna_n)
                        for c in range(2):
                            tr128(lnaT[c][:, tb * 128:(tb + 1) * 128],
                                  lna_n[:, c * 128:(c + 1) * 128])

                # --- MLP over N quarters ---
                upd_sums = {}
                for q in range(NQ):
                    qs = slice(q * (N // NQ), (q + 1) * (N // NQ))
                    wxq = wq.tile([128, 2, N // NQ], F32R, name="wxq", tag="wxq")
                    nc.sync.dma_start(wxq[:], wx_r[:, :, qs])
                    wyq = wq.tile([128, 2, N // NQ], F32R, name="wyq", tag="wyq")
                    nc.sync.dma_start(wyq[:], wy_r[:, :, qs])
                    encq = wq.tile([128, NCHQ, D], F32R, name="encq", tag="encq")
                    nc.sync.dma_start(encq[:], enc_r[:, q * NCHQ:(q + 1) * NCHQ, :])
                    for si in range(NSUP):
                        sl = slice(si * TS, (si + 1) * TS)
                        ln_src = lnaT if attn else qrT
                        psU = [ps256.tile([128, D], F32, name="psU", tag="ps256")
                               for _ in range(4)]
                        for nch in range(NCHQ):
                            psX = ps512.tile([128, TS], F32, name="psX", tag="ps512")
                            psY = ps512.tile([128, TS], F32, name="psY", tag="ps512")
                            ns = slice(nch * 128, (nch + 1) * 128)
                            for i, (wt, act) in enumerate(((wxq, vT), (wyq, ln_src))):
                                ps = psX if i == 0 else psY
                                for c in range(2):
                                    nc.tensor.matmul(ps, wt[:, c, ns], act[c][:, sl],
                                                     start=(c == 0), stop=(c == 1))
                            xr = blkp.tile([128, TS], F32, name="xr", tag="blk")
                            nc.scalar.activation(xr, psX, ACTF.Relu)
                            ysb = blkp.tile([128, TS], F32R, name="ysb", tag="blk")
                            nc.vector.scalar_tensor_tensor(
                                ysb, psY, 0.0, xr, op0=ALU.max, op1=ALU.mult)
                            for tb4 in range(4):
                                t4 = slice(tb4 * 128, (tb4 + 1) * 128)
                                nc.tensor.matmul(
                                    psU[tb4], ysb[:, t4], encq[:, nch, :],
                                    start=(nch == 0), stop=(nch == NCHQ - 1))
                        for tb4 in range(4):
                            tb = si * 4 + tb4
                            dst = updA(tb)
                            if q == 0:
                                nc.scalar.copy(dst, psU[tb4])
                            elif q < NQ - 1:
                                nc.vector.tensor_tensor(dst, psU[tb4], dst, op=ALU.add)
                            else:
                                s2 = stp.tile([128, 1], F32, name="s2", tag="st")
                                nc.vector.scalar_tensor_tensor(
                                    dst, psU[tb4], 0.0, dst, op0=ALU.add, op1=ALU.add,
                                    accum_out=s2)
                                upd_sums[tb] = s2

                # --- v = ln(v + ln(update)); maintain both layouts ---
                for tb in range(NTB if cphase else 0):
                    upd = updA(tb)
                    lnu = scp.tile([128, D], F32, name="lnu", tag="sc")
                    ln_nat(upd, lnu, sums=upd_sums[tb])
                    vmid = scp.tile([128, D], F32, name="vmid", tag="sc")
                    s3 = stp.tile([128, 1], F32, name="s3", tag="st")
                    nc.vector.scalar_tensor_tensor(
                        vmid, lnu, 0.0, vN[:, tb, :], op0=ALU.add, op1=ALU.add,
                        accum_out=s3)
                    ln_nat(vmid, vN[:, tb, :], sums=s3)
                    for c in range(2):
                        tr128(vT[c][:, tb * 128:(tb + 1) * 128],
                              vN[:, tb, c * 128:(c + 1) * 128])

            # ---------------- readout ----------------
            ro_s = blkp.tile([128, 2, D], F32R, name="ro_s", tag="blk")
            nc.sync.dma_start(ro_s[:], ro_r)
            for tb in range(NTB):
                psR = ps256.tile([128, D], F32, name="psR", tag="ps256")
                for c in range(2):
                    nc.tensor.matmul(psR, vT[c][:, tb * 128:(tb + 1) * 128],
                                     ro_s[:, c, :], start=(c == 0), stop=(c == 1))
                lo = scp.tile([128, VOCAB], F32, name="lo", tag="sc")
                copy_any(lo[:], psR[:])
                nc.sync.dma_start(out_d.ap()[tb * 128:(tb + 1) * 128, :], lo[:])

    nc.compile()
    return nc


_NC_CACHE = {}


def get_nc():
    if "nc" not in _NC_CACHE:
        _NC_CACHE["nc"] = build_nc()
    return _NC_CACHE["nc"]


def make_host_inputs(idx, wte, encoder, decoder_x, decoder_y, readout):
    idx = np.asarray(idx)
    wte = np.asarray(wte, dtype=np.float32)
    encoder = np.asarray(encoder, dtype=np.float32)
    decoder_x = np.asarray(decoder_x, dtype=np.float32)
    decoder_y = np.asarray(decoder_y, dtype=np.float32)
    readout = np.asarray(readout, dtype=np.float32)

    wx = decoder_x.transpose(1, 0, 2).reshape(D, N)
    wy = decoder_y.transpose(1, 0, 2).reshape(D, N)
    # partition-contiguous layouts for fast DMA: [p, c, n] with d = c*128 + p
    wx = np.ascontiguousarray(wx.reshape(2, 128, N).transpose(1, 0, 2))
    wy = np.ascontiguousarray(wy.reshape(2, 128, N).transpose(1, 0, 2))
    # enc: [p, o, d] with n = o*128 + p
    enc_s = np.ascontiguousarray(encoder.reshape(N // 128, 128, D).transpose(1, 0, 2))

    com = {"wx": wx, "wy": wy, "enc": enc_s}

    inv_freq = 1.0 / (10000.0 ** (np.arange(0, D, 2, dtype=np.float32) / D))  # [128]
    t = np.arange(T, dtype=np.float32)
    freqsT = inv_freq[:, None] * t[None, :]                   # [128, T]
    com["cosT"] = np.cos(freqsT).astype(np.float32)
    com["sinT"] = np.sin(freqsT).astype(np.float32)

    s_idx = np.arange(128, dtype=np.int32)[:, None]
    c_idx = np.arange(1024, dtype=np.int32)[None, :]
    com["maskbig"] = (s_idx <= c_idx - 384).astype(ml_dtypes.bfloat16)
    com["wte"] = wte
    com["ro"] = readout
    com["identm"] = np.eye(128, dtype=np.float32)

    in_maps = []
    for b in range(B):
        m = dict(com)
        m["idxf"] = idx[b].astype(np.float32).reshape(1, T)
        in_maps.append(m)
    return in_maps


def kernel(idx, wte, encoder, decoder_x, decoder_y, readout):
    nc = get_nc()
    in_maps = make_host_inputs(idx, wte, encoder, decoder_x, decoder_y, readout)
    res = bass_utils.run_bass_kernel_spmd(nc, in_maps, core_ids=list(range(B)))
    out = np.stack([res.results[b]["logits"] for b in range(B)], axis=0)
    return out.astype(np.float32)
